# revision 2
# baseline (speedup 1.0000x reference)
"""MoE FFN (dense expert-parallel) Trainium2 kernel.

Strategy: expert-parallel across 8 NeuronCores. Core e holds expert e's
FFN weights and computes, for ALL 8192 tokens: the gate (fp32, on device),
its expert's FFN output (bf16 matmuls, fp32 accumulate), and scales by the
top-2-renormalized gate weight of its expert (zero when not selected).
Host sums the 8 partial outputs (the "psum" combine of the unshard step).

The per-core expert selection is encoded purely in the input layout: each
core receives Wg/bg with expert columns permuted so its own expert is
column 0 — the gate math is permutation-equivariant, so column 0 of the
weight matrix is always "my expert".
"""
import sys
import types

sys.path.insert(0, "/opt/trn_rl_repo")

import numpy as np
import ml_dtypes

import bass_rust
import concourse.bass as bass
import concourse.mybir as mybir
import concourse.bass_utils as bu
from concourse.tile import TileContext

BF16 = ml_dtypes.bfloat16

B, T, C, E, H = 4, 2048, 1024, 8, 4096
NT = B * T          # 8192 tokens
P = 128
KC = C // P         # 8 k-tiles over C
KH = H // P         # 32 k-tiles over H
TOKCH = 512         # tokens per FFN chunk
NCH = NT // TOKCH   # 16 chunks
NG = NT // P        # 64 gate tiles
CCH = C // 512      # 2 output column chunks

F32 = mybir.dt.float32
BF = mybir.dt.bfloat16
Relu = mybir.ActivationFunctionType.Relu
Exp = mybir.ActivationFunctionType.Exp


def _split_excess_waits(nc):
    """walrus codegen allows 1 sem-wait per instruction (2 on
    EventSemaphore). Move excess waits onto same-engine EventSemaphore
    insts placed just before (engine program order preserves semantics)."""
    for f in nc.m.functions:
        for bb in f.blocks:
            new = []
            changed = False
            for inst in bb.instructions:
                si = inst.sync_info
                cap = 2 if isinstance(inst, mybir.InstEventSemaphore) else 1
                if si is not None and len(si.on_wait) > cap:
                    waits = list(si.on_wait)
                    extra, keep = waits[:-cap], waits[-cap:]
                    for i in range(0, len(extra), 2):
                        w = mybir.InstEventSemaphore(
                            name=f"{inst.name}_presem{i}", ins=[], outs=[])
                        w.engine = inst.engine
                        w.sync_info = bass_rust.SyncInfo(
                            on_wait=extra[i:i + 2], on_update=[])
                        new.append(w)
                        changed = True
                    inst.sync_info = bass_rust.SyncInfo(
                        on_wait=keep, on_update=list(si.on_update))
                new.append(inst)
            if changed:
                bb.instructions = new


def _build_dense():
    nc = bass.Bass()
    xt = nc.declare_dram_parameter("xt", [C, NT], F32, isOutput=False)
    xtb = nc.declare_dram_parameter("xtb", [C, NT], BF, isOutput=False)
    w1 = nc.declare_dram_parameter("w1", [C, H], BF, isOutput=False)
    b1c = nc.declare_dram_parameter("b1c", [P, KH], F32, isOutput=False)
    w2 = nc.declare_dram_parameter("w2", [H, C], BF, isOutput=False)
    b2r = nc.declare_dram_parameter("b2r", [1, C], BF, isOutput=False)
    wgp = nc.declare_dram_parameter("wgp", [C, E], F32, isOutput=False)
    bgp = nc.declare_dram_parameter("bgp", [1, E], F32, isOutput=False)
    out = nc.declare_dram_parameter("out", [NT, C], F32, isOutput=True)

    with TileContext(nc) as tc:
        with tc.tile_pool(name="wpool", bufs=1) as wpool, \
             tc.tile_pool(name="gpool", bufs=4) as gpool, \
             tc.tile_pool(name="xgpool", bufs=2) as xgpool, \
             tc.tile_pool(name="xbpool", bufs=1) as xbpool, \
             tc.tile_pool(name="htpool", bufs=1) as htpool, \
             tc.tile_pool(name="ypool", bufs=3) as ypool, \
             tc.tile_pool(name="psg", bufs=2, space="PSUM") as psgp, \
             tc.tile_pool(name="ps1", bufs=2, space="PSUM") as ps1p, \
             tc.tile_pool(name="ps2", bufs=2, space="PSUM") as ps2p:

            # ---- resident weights / constants
            w1_sb = []
            for k in range(KC):
                t = wpool.tile([P, H], BF, tag=f"w1k{k}")
                nc.sync.dma_start(out=t[:], in_=w1[k * P:(k + 1) * P, :])
                w1_sb.append(t)
            w2_sb = []
            for h in range(KH):
                t = wpool.tile([P, C], BF, tag=f"w2k{h}")
                nc.sync.dma_start(out=t[:], in_=w2[h * P:(h + 1) * P, :])
                w2_sb.append(t)
            wg_sb = []
            for k in range(KC):
                t = wpool.tile([P, E], F32, tag=f"wgk{k}")
                nc.sync.dma_start(out=t[:], in_=wgp[k * P:(k + 1) * P, :])
                wg_sb.append(t)
            b1c_sb = wpool.tile([P, KH], F32, tag="b1c")
            nc.sync.dma_start(out=b1c_sb[:], in_=b1c[:])
            b2r_sb = wpool.tile([1, C], BF, tag="b2r")
            nc.sync.dma_start(out=b2r_sb[:], in_=b2r[:])
            bg_sb = wpool.tile([1, E], F32, tag="bgp")
            nc.sync.dma_start(out=bg_sb[:], in_=bgp[:])
            ones_f = wpool.tile([1, P], F32, tag="ones_f")
            nc.vector.memset(ones_f[:], 1.0)
            ones_b = wpool.tile([1, P], BF, tag="ones_b")
            nc.vector.memset(ones_b[:], 1.0)
            # per-token gate weight of "my" expert, column g = token tile g
            wcol = wpool.tile([P, NG], F32, tag="wcol")

            # ---- gate phase: fp32 logits -> softmax -> top2 renorm weight
            for g in range(NG):
                xg = [xgpool.tile([P, P], F32, tag=f"xg{k}", name=f"xg{k}")
                      for k in range(KC)]
                for k in range(KC):
                    nc.sync.dma_start(
                        out=xg[k][:],
                        in_=xt[k * P:(k + 1) * P, g * P:(g + 1) * P])
                psg = psgp.tile([P, E], F32)
                for k in range(KC):
                    nc.tensor.matmul(out=psg[:], lhsT=xg[k][:], rhs=wg_sb[k][:],
                                     start=(k == 0), stop=False)
                nc.tensor.matmul(out=psg[:], lhsT=ones_f[:], rhs=bg_sb[:],
                                 start=False, stop=True)
                m = gpool.tile([P, 1], F32, tag="gm")
                nc.vector.reduce_max(out=m[:], in_=psg[:],
                                     axis=mybir.AxisListType.X)
                nm = gpool.tile([P, 1], F32, tag="gnm")
                nc.vector.tensor_scalar_mul(nm[:], m[:], -1.0)
                pexp = gpool.tile([P, E], F32, tag="gpexp")
                nc.scalar.activation(pexp[:], psg[:], Exp, bias=nm[:])
                s = gpool.tile([P, 1], F32, tag="gs")
                nc.vector.reduce_sum(out=s[:], in_=pexp[:],
                                     axis=mybir.AxisListType.X)
                rs = gpool.tile([P, 1], F32, tag="grs")
                nc.vector.reciprocal(rs[:], s[:])
                pn = gpool.tile([P, E], F32, tag="gpn")
                nc.vector.tensor_scalar_mul(pn[:], pexp[:], rs[:])
                top8 = gpool.tile([P, E], F32, tag="gtop8")
                nc.vector.max(out=top8[:], in_=pn[:])
                etop = gpool.tile([P, 2], F32, tag="getop")
                nc.scalar.activation(etop[:], top8[:, 0:2], Exp)
                d = gpool.tile([P, 1], F32, tag="gd")
                nc.vector.reduce_sum(out=d[:], in_=etop[:],
                                     axis=mybir.AxisListType.X)
                rd = gpool.tile([P, 1], F32, tag="grd")
                nc.vector.reciprocal(rd[:], d[:])
                ep0 = gpool.tile([P, 1], F32, tag="gep0")
                nc.scalar.activation(ep0[:], pn[:, 0:1], Exp)
                mask0 = gpool.tile([P, 1], F32, tag="gmask0")
                nc.vector.tensor_tensor(out=mask0[:], in0=pn[:, 0:1],
                                        in1=top8[:, 1:2],
                                        op=mybir.AluOpType.is_ge)
                t1 = gpool.tile([P, 1], F32, tag="gt1")
                nc.vector.tensor_tensor(out=t1[:], in0=ep0[:], in1=mask0[:],
                                        op=mybir.AluOpType.mult)
                nc.vector.tensor_tensor(out=wcol[:, g:g + 1], in0=t1[:],
                                        in1=rd[:], op=mybir.AluOpType.mult)

            # ---- FFN phase
            for q in range(NCH):
                xb = [xbpool.tile([P, TOKCH], BF, tag=f"xb{k}", name=f"xb{k}")
                      for k in range(KC)]
                for k in range(KC):
                    nc.sync.dma_start(
                        out=xb[k][:],
                        in_=xtb[k * P:(k + 1) * P,
                                q * TOKCH:(q + 1) * TOKCH])
                # h^T = relu(W1^T x + b1): [H, tok]
                ht = []
                for h in range(KH):
                    ps1 = ps1p.tile([P, TOKCH], F32)
                    for k in range(KC):
                        nc.tensor.matmul(
                            out=ps1[:],
                            lhsT=w1_sb[k][:, h * P:(h + 1) * P],
                            rhs=xb[k][:],
                            start=(k == 0), stop=(k == KC - 1))
                    htt = htpool.tile([P, TOKCH], BF, tag=f"ht{h}")
                    nc.scalar.activation(htt[:], ps1[:], Relu,
                                         bias=b1c_sb[:, h:h + 1])
                    ht.append(htt)
                # y = h^T.T @ W2 + b2, scaled by gate weight: [tok, C]
                for tt in range(TOKCH // P):
                    g = q * (TOKCH // P) + tt
                    for cc in range(CCH):
                        ps2 = ps2p.tile([P, 512], F32)
                        for h in range(KH):
                            nc.tensor.matmul(
                                out=ps2[:],
                                lhsT=ht[h][:, tt * P:(tt + 1) * P],
                                rhs=w2_sb[h][:, cc * 512:(cc + 1) * 512],
                                start=(h == 0), stop=False)
                        nc.tensor.matmul(
                            out=ps2[:], lhsT=ones_b[:],
                            rhs=b2r_sb[:, cc * 512:(cc + 1) * 512],
                            start=False, stop=True)
                        y = ypool.tile([P, 512], F32, tag="y")
                        nc.vector.tensor_scalar_mul(y[:], ps2[:],
                                                    wcol[:, g:g + 1])
                        nc.sync.dma_start(
                            out=out[g * P:(g + 1) * P,
                                    cc * 512:(cc + 1) * 512],
                            in_=y[:])

    _split_excess_waits(nc)
    return nc


_NC_CACHE = {}


def _get_nc():
    if "dense" not in _NC_CACHE:
        _NC_CACHE["dense"] = _build_dense()
    return _NC_CACHE["dense"]


def _prep_inputs(x, W1, b1, W2, b2, Wg, bg):
    xf = np.ascontiguousarray(np.asarray(x, dtype=np.float32).reshape(NT, C))
    xt = np.ascontiguousarray(xf.T)
    xtb = xt.astype(BF16)
    in_maps = []
    for e in range(E):
        perm = [e] + [i for i in range(E) if i != e]
        in_maps.append({
            "xt": xt,
            "xtb": xtb,
            "w1": np.ascontiguousarray(W1[e]).astype(BF16),
            "b1c": np.ascontiguousarray(b1[e].reshape(KH, P).T),
            "w2": np.ascontiguousarray(W2[e]).astype(BF16),
            "b2r": np.ascontiguousarray(b2[e].reshape(1, C)).astype(BF16),
            "wgp": np.ascontiguousarray(Wg[:, perm]).astype(np.float32),
            "bgp": np.ascontiguousarray(bg[perm].reshape(1, E)).astype(
                np.float32),
        })
    return in_maps


def run(x, W1, b1, W2, b2, Wg, bg, trace=False, tmpdir=None):
    nc = _get_nc()
    in_maps = _prep_inputs(x, W1, b1, W2, b2, Wg, bg)
    res = bu.run_bass_kernel_spmd(nc, in_maps, list(range(E)), trace=trace,
                                  tmpdir=tmpdir)
    acc = res.results[0]["out"].astype(np.float32)
    for e in range(1, E):
        acc += res.results[e]["out"]
    return acc.reshape(B, T, C), res


def kernel(x, W1, b1, W2, b2, Wg, bg):
    out, _ = run(x, W1, b1, W2, b2, Wg, bg)
    return out
